# revision 1
# baseline (speedup 1.0000x reference)
"""Trainium2 Bass kernel for nn_Critic (dense MLP critic, 4 layers + LayerNorms).

Strategy (pure data parallel over 8 NeuronCores):
  - batch B=32768 sharded 8x -> 4096 rows/core; weights replicated.
  - all activations kept feature-major ([features on partitions, batch on
    free dim]) so the contraction dim of every matmul is the partition dim.
  - LayerNorm folded into the matmuls: for y = LN(z; g, beta) @ W.T + b,
      y[o,b] = invs[b]*( (W*g)z[o,b] - mu[b]*rowsum(W*g)[o] ) + (W@beta+b)[o]
    realized as an augmented matmul: activations get two extra K-rows
    (-mu[b], sigma[b]) and the weight matrix two extra rows
    (rowsum(W*g)[o], (W@beta+b)[o]); then h = tanh(invs (.) psum).
  - L1 stats (mean/var over 2080 features) via bn_stats on a second,
    batch-major copy of z; transposed to rows via a tiny PE transpose.
  - L2/L3 stats via (+-1/512)-ones-vector matmuls on PE (sum and sum-of-
    squares of h), with h^2 from ACT Square.
  - fp16 data everywhere (weights, activations), f32 PSUM/statistics.
"""

import os
import sys
import numpy as np

for _p in ("/opt/trn_rl_repo",):
    if os.path.isdir(_p) and _p not in sys.path:
        sys.path.append(_p)

from contextlib import ExitStack

import concourse.bass as bass  # noqa: E402
import concourse.tile as tile  # noqa: E402
from concourse import bacc, mybir  # noqa: E402
from concourse.bass_utils import run_bass_kernel_spmd  # noqa: E402

NCORES = 8
B = 32768
BC = B // NCORES  # rows per core
INPUT_DIM = 2048
HALF = INPUT_DIM // 2
N_ACTIONS = 32
D = INPUT_DIM + N_ACTIONS  # 2080
H = 512
NT = 512  # batch columns per tile
EPS = 1e-5
X_NORM = 50.0
V_NORM = 10.0

F16 = mybir.dt.float16
F32 = mybir.dt.float32
AF = mybir.ActivationFunctionType

K1 = 17  # ceil(D/128); last chunk has 32 data rows + 2 aug rows
K1_LAST = D - 16 * 128  # 32


def build_nc(bout: float, bc: int = BC):
    """Build + compile the per-core program. bc = rows per core."""
    ntiles = bc // NT
    assert ntiles * NT == bc

    nc = bacc.Bacc("TRN2", target_bir_lowering=False, debug=False,
                   num_devices=NCORES)

    zr_d = nc.dram_tensor("zr", [bc, D], F16, kind="ExternalInput").ap()
    zt_d = nc.dram_tensor("zt", [D, bc], F16, kind="ExternalInput").ap()
    w1_d = nc.dram_tensor("w1a", [D + 2, H], F16, kind="ExternalInput").ap()
    w2_d = nc.dram_tensor("w2a", [H + 2, H], F16, kind="ExternalInput").ap()
    w3_d = nc.dram_tensor("w3a", [H + 2, H], F16, kind="ExternalInput").ap()
    wo_d = nc.dram_tensor("wout", [H, 1], F16, kind="ExternalInput").ap()
    id_d = nc.dram_tensor("ident", [128, 128], F32, kind="ExternalInput").ap()
    q_d = nc.dram_tensor("q", [1, bc], F32, kind="ExternalOutput").ap()

    with tile.TileContext(nc) as tc:
        _emit(tc, ntiles, bout, zr_d, zt_d, w1_d, w2_d, w3_d, wo_d, id_d, q_d)

    nc.compile()
    return nc


def _emit(tc, ntiles, bout, zr_d, zt_d, w1_d, w2_d, w3_d, wo_d, id_d, q_d):
    nc = tc.nc
    with ExitStack() as ctx:
        wp = ctx.enter_context(tc.tile_pool(name="wp", bufs=1))
        zt_p = ctx.enter_context(tc.tile_pool(name="ztp", bufs=2))
        zr_p = ctx.enter_context(tc.tile_pool(name="zrp", bufs=2))
        h_p = ctx.enter_context(tc.tile_pool(name="hp", bufs=2))
        u_p = ctx.enter_context(tc.tile_pool(name="up", bufs=3))
        sq_p = ctx.enter_context(tc.tile_pool(name="sqp", bufs=3))
        bc_p = ctx.enter_context(tc.tile_pool(name="bcp", bufs=2))
        st_p = ctx.enter_context(tc.tile_pool(name="stp", bufs=3))
        ps_y = ctx.enter_context(tc.tile_pool(name="psy", bufs=3, space="PSUM"))
        ps_s = ctx.enter_context(tc.tile_pool(name="pss", bufs=1, space="PSUM"))
        ps_t = ctx.enter_context(tc.tile_pool(name="pst", bufs=2, space="PSUM"))
        ps_q = ctx.enter_context(tc.tile_pool(name="psq", bufs=1, space="PSUM"))

        # ---- persistent constants / weights ----
        w1 = []
        for k in range(K1):
            rows = 128 if k < 16 else K1_LAST + 2
            t = wp.tile([rows, H], F16, tag=f"w1_{k}")
            nc.sync.dma_start(out=t[:, :], in_=w1_d[k * 128:k * 128 + rows, :])
            w1.append(t)
        w2 = []
        w3 = []
        for name, wd, lst in (("w2", w2_d, w2), ("w3", w3_d, w3)):
            for k in range(4):
                t = wp.tile([128, H], F16, tag=f"{name}_{k}")
                nc.sync.dma_start(out=t[:, :], in_=wd[k * 128:(k + 1) * 128, :])
                lst.append(t)
            # rows H (rs) and H+1 (c) as separate [1, H] tiles
            for j in range(2):
                t = wp.tile([1, H], F16, tag=f"{name}_aug{j}")
                nc.sync.dma_start(out=t[:, :], in_=wd[H + j:H + j + 1, :])
                lst.append(t)
        wo = wp.tile([128, 4], F16, tag="wo")
        for k in range(4):
            nc.sync.dma_start(out=wo[:, k:k + 1], in_=wo_d[k * 128:(k + 1) * 128, :])
        ident = wp.tile([128, 128], F32, tag="ident")
        nc.sync.dma_start(out=ident[:, :], in_=id_d[:, :])
        onesn = wp.tile([128, 1], F16, tag="onesn")
        nc.vector.memset(onesn[:, :], -1.0 / H)
        onesp = wp.tile([128, 1], F16, tag="onesp")
        nc.vector.memset(onesp[:, :], 1.0 / H)
        epsT = wp.tile([128, 1], F32, tag="epsT")
        nc.vector.memset(epsT[:, :], EPS)
        boutT = wp.tile([1, 1], F32, tag="boutT")
        nc.vector.memset(boutT[:, :], bout)
        qrow = wp.tile([1, ntiles * NT], F32, tag="qrow")

        def evac(py, bctile, htile):
            """h = tanh(invs (.) psum) : DVE multiply + ACT tanh."""
            u = u_p.tile([128, NT], F16, tag="u")
            nc.vector.tensor_mul(u[:, :], py[:, :], bctile[:, :])
            nc.scalar.activation(htile[:, :], u[:, :], AF.Tanh)

        def bcast(row_ap):
            t = bc_p.tile([128, NT], F32, tag="bc")
            nc.gpsimd.partition_broadcast(t[:, :], row_ap)
            return t

        for it in range(ntiles):
            bs = it * NT

            # ---- L1 stats: bn_stats over batch-major z rows ----
            zt16 = zt_p.tile([K1_LAST + 2, NT], F16, tag="zt16")
            invs1 = st_p.tile([1, NT], F32, tag="invs1")
            zrt = zr_p.tile([128, 4, D], F16, tag="zrall")
            nc.sync.dma_start(out=zrt[:, :, :],
                              in_=zr_d[bs:bs + NT, :].rearrange("(c p) d -> p c d", c=4))
            for bch in range(4):
                stats = st_p.tile([128, 5, 6], F32, tag=f"st{bch}")
                zrv = zrt[:, bch, :].rearrange("p (n s) -> p n s", n=5)
                for i in range(5):
                    nc.vector.bn_stats(out=stats[:, i, :], in_=zrv[:, i, :])
                mv = st_p.tile([128, 2], F32, tag=f"mv{bch}")
                nc.vector.bn_aggr(out=mv[:, :], in_=stats[:, :, :])
                # pt cols: [sigma, -mu]; transposed rows pair with w1a aug
                # rows (c1, rs1) in that order.
                pt = st_p.tile([128, 2], F32, tag=f"pt{bch}")
                nc.scalar.activation(pt[:, 0:1], mv[:, 1:2], AF.Sqrt, bias=epsT[:, :])
                nc.vector.tensor_scalar_mul(pt[:, 1:2], mv[:, 0:1], -1.0)
                ptr = ps_t.tile([2, 128], F32, tag="ptr")
                nc.tensor.transpose(out=ptr[:, :], in_=pt[:, :], identity=ident[:, :])
                sl = slice(bch * 128, (bch + 1) * 128)
                nc.vector.tensor_copy(out=zt16[K1_LAST:K1_LAST + 2, sl], in_=ptr[0:2, :])
                nc.vector.reciprocal(invs1[0:1, sl], ptr[0:1, :])

            # ---- zT loads: one strided DMA for the 16 full chunks ----
            ztmain = zt_p.tile([128, 16, NT], F16, tag="ztmain")
            nc.sync.dma_start(
                out=ztmain[:, :, :],
                in_=zt_d[0:2048, bs:bs + NT].rearrange("(k p) n -> p k n", k=16))
            nc.sync.dma_start(out=zt16[0:K1_LAST, :], in_=zt_d[2048:2048 + K1_LAST, bs:bs + NT])
            zts = [ztmain[:, k, :] for k in range(16)] + [zt16]

            # ---- L1 matmuls + evac ----
            bc1 = bcast(invs1[0:1, :])
            h1 = []
            for m in range(4):
                py = ps_y.tile([128, NT], F32, tag="py")
                msl = slice(m * 128, (m + 1) * 128)
                for k in range(K1):
                    rk = zts[k] if k < 16 else zts[k][:, :]
                    nc.tensor.matmul(py[:, :], lhsT=w1[k][:, msl], rhs=rk,
                                     start=(k == 0), stop=(k == K1 - 1))
                ht = h_p.tile([128, NT], F16, tag=f"h1_{m}")
                evac(py, bc1, ht)
                h1.append(ht)

            # ---- L2 / L3 ----
            hcur = h1
            for lname, wts in (("l2", w2), ("l3", w3)):
                # stats: s1 = -mean, s2 = +E[h^2]
                s1 = ps_s.tile([1, NT], F32, tag="s1")
                s2 = ps_s.tile([1, NT], F32, tag="s2")
                for k in range(4):
                    nc.tensor.matmul(s1[:, :], lhsT=onesn[:, :], rhs=hcur[k][:, :],
                                     start=(k == 0), stop=(k == 3))
                for k in range(4):
                    sq = sq_p.tile([128, NT], F16, tag="sq")
                    nc.vector.tensor_mul(sq[:, :], hcur[k][:, :], hcur[k][:, :])
                    nc.tensor.matmul(s2[:, :], lhsT=onesp[:, :], rhs=sq[:, :],
                                     start=(k == 0), stop=(k == 3))
                musq = st_p.tile([1, NT], F32, tag="musq")
                nc.scalar.square(musq[:, :], s1[:, :])
                varr = st_p.tile([1, NT], F32, tag="var")
                nc.vector.tensor_sub(varr[:, :], s2[:, :], musq[:, :])
                negmu = h_p.tile([1, NT], F16, tag=f"negmu_{lname}")
                nc.vector.tensor_copy(out=negmu[:, :], in_=s1[:, :])
                sig32 = st_p.tile([1, NT], F32, tag="sig32")
                nc.scalar.activation(sig32[:, :], varr[:, :], AF.Sqrt, bias=epsT[0:1, :])
                sig16 = h_p.tile([1, NT], F16, tag=f"sig16_{lname}")
                nc.vector.tensor_copy(out=sig16[:, :], in_=sig32[:, :])
                invs = st_p.tile([1, NT], F32, tag="invs")
                nc.vector.reciprocal(invs[:, :], sig32[:, :])
                bct = bcast(invs[0:1, :])
                hnew = []
                for m in range(4):
                    py = ps_y.tile([128, NT], F32, tag="py")
                    msl = slice(m * 128, (m + 1) * 128)
                    for k in range(4):
                        nc.tensor.matmul(py[:, :], lhsT=wts[k][:, msl], rhs=hcur[k][:, :],
                                         start=(k == 0), stop=False)
                    nc.tensor.matmul(py[:, :], lhsT=wts[4][:, msl], rhs=negmu[:, :],
                                     start=False, stop=False)
                    nc.tensor.matmul(py[:, :], lhsT=wts[5][:, msl], rhs=sig16[:, :],
                                     start=False, stop=True)
                    ht = h_p.tile([128, NT], F16, tag=f"h_{lname}_{m}")
                    evac(py, bct, ht)
                    hnew.append(ht)
                hcur = hnew

            # ---- L4 ----
            pq = ps_q.tile([1, NT], F32, tag="pq")
            for k in range(4):
                nc.tensor.matmul(pq[:, :], lhsT=wo[:, k:k + 1], rhs=hcur[k][:, :],
                                 start=(k == 0), stop=(k == 3))
            nc.scalar.activation(qrow[0:1, bs:bs + NT], pq[:, :], AF.Tanh, bias=boutT[:, :])

        nc.sync.dma_start(out=q_d[:, :], in_=qrow[:, :])


# ---------------- host side ----------------

def host_prep(x, a, g1, beta1, g2, beta2, g3, beta3,
              w1, b1, w2, b2, w3, b3, w_out, b_out):
    """Shared (replicated) tensors + full z arrays; returns dict pieces."""
    f16 = np.float16
    z = np.empty((x.shape[0], D), dtype=f16)
    np.multiply(x[:, :HALF], np.float32(1.0 / X_NORM), out=z[:, :HALF], casting="unsafe")
    np.multiply(x[:, HALF:], np.float32(1.0 / V_NORM), out=z[:, HALF:INPUT_DIM], casting="unsafe")
    z[:, INPUT_DIM:] = a.astype(f16)

    def fold(w, g, beta, b, sigma_first):
        wg = (w.astype(np.float64) * g.astype(np.float64)[None, :])
        rs = wg.sum(axis=1)
        c = w.astype(np.float64) @ beta.astype(np.float64) + b.astype(np.float64)
        out = np.empty((w.shape[1] + 2, w.shape[0]), dtype=f16)
        out[:w.shape[1]] = wg.T.astype(f16)
        # L1 device aug rows arrive as (sigma, -mu) -> weight rows (c, rs);
        # L2/L3 use separate (negmu, sigma) rhs -> weight rows (rs, c).
        first, second = (c, rs) if sigma_first else (rs, c)
        out[w.shape[1]] = first.astype(f16)
        out[w.shape[1] + 1] = second.astype(f16)
        return out

    w1a = fold(w1, g1, beta1, b1, True)
    w2a = fold(w2, g2, beta2, b2, False)
    w3a = fold(w3, g3, beta3, b3, False)
    wout = w_out.T.astype(f16)  # [H, 1]
    bout = float(b_out[0])
    ident = np.eye(128, dtype=np.float32)
    return z, w1a, w2a, w3a, wout, bout, ident


_NC_CACHE = {}


def kernel(**inputs):
    inputs = {k: np.asarray(v) for k, v in inputs.items()}
    z, w1a, w2a, w3a, wout, bout, ident = host_prep(**inputs)

    key = (round(bout, 10), BC)
    if key not in _NC_CACHE:
        _NC_CACHE[key] = build_nc(bout, BC)
    nc = _NC_CACHE[key]

    in_maps = []
    for c in range(NCORES):
        zc = z[c * BC:(c + 1) * BC]
        in_maps.append({
            "zr": np.ascontiguousarray(zc),
            "zt": np.ascontiguousarray(zc.T),
            "w1a": w1a, "w2a": w2a, "w3a": w3a, "wout": wout, "ident": ident,
        })

    res = run_bass_kernel_spmd(nc, in_maps, list(range(NCORES)))
    q = np.concatenate([res.results[c]["q"].reshape(BC, 1) for c in range(NCORES)],
                       axis=0).astype(np.float32)
    return q



# revision 3
# speedup vs baseline: 83.4012x; 83.4012x over previous
"""Trainium2 Bass kernel for nn_Critic (dense MLP critic, 4 layers + LayerNorms).

v2 strategy (pure data parallel over 8 NeuronCores, batch 8x4096):
  - activations feature-major ([feat on partitions, batch on free dim]);
    weights stationary; PSUM accumulation over K chunks.
  - LayerNorm folded into matmuls:
      y = LN(z;g,beta) @ W.T + b
        = invs (.) [ (Wg)z - mu * rowsum(Wg) ] + (W@beta + b)
    realized as: K-chunk matmuls + one aug row (-mu) paired with the
    rowsum weight row, then h = ACT Tanh(invs*psum, bias=c_col) where
    c_col = (W@beta+b) is a per-partition bias column (free on ACT).
  - ACT issues ONLY Tanh/Square -> single activation table, no reloads.
  - rsqrt(var+eps) on DVE: Quake-III bit seed + 1 Newton step.
  - L1 stats (2080 feats) via DVE bn_stats on a batch-major copy of z;
    invs/-mu flipped to rows by one PE transpose; invs broadcast to a
    [128,NT] PSUM tile by a K=1 ones matmul.
  - L2/L3 stats via ones[128,128] matmuls: -mean / E[h^2] arrive already
    broadcast across partitions in PSUM; h^2 from ACT Square.
  - fp16 data, f32 PSUM/stats.
"""

import os
import sys
import numpy as np

for _p in ("/opt/trn_rl_repo",):
    if os.path.isdir(_p) and _p not in sys.path:
        sys.path.append(_p)

from contextlib import ExitStack

import concourse.bass as bass  # noqa: E402
import concourse.tile as tile  # noqa: E402
from concourse import bacc, mybir  # noqa: E402
from concourse.bass_utils import run_bass_kernel_spmd  # noqa: E402

NCORES = 8
B = 32768
BC = B // NCORES
INPUT_DIM = 2048
HALF = INPUT_DIM // 2
N_ACTIONS = 32
D = INPUT_DIM + N_ACTIONS  # 2080
H = 512
NT = 512
EPS = 1e-5
X_NORM = 50.0
V_NORM = 10.0

F16 = mybir.dt.float16
F32 = mybir.dt.float32
I32 = mybir.dt.int32
AF = mybir.ActivationFunctionType
OP = mybir.AluOpType

K1 = 17               # ceil(D/128); last chunk: 32 data rows + 1 aug row
K1_LAST = D - 16 * 128  # 32
MAGIC = 0x5F3759DF
SEPS = float(np.sqrt(EPS))  # eps injected into s2b as SEPS*SEPS via matmul


def build_nc(bout: float, bc: int = BC, newton2: bool = False):
    ntiles = bc // NT
    assert ntiles * NT == bc

    nc = bacc.Bacc("TRN2", target_bir_lowering=False, debug=False,
                   num_devices=NCORES)

    zr_d = nc.dram_tensor("zr", [bc, D], F16, kind="ExternalInput").ap()
    zt_d = nc.dram_tensor("zt", [D, bc], F16, kind="ExternalInput").ap()
    w1_d = nc.dram_tensor("w1a", [D + 1, H], F16, kind="ExternalInput").ap()
    w2_d = nc.dram_tensor("w2a", [H + 1, H], F16, kind="ExternalInput").ap()
    w3_d = nc.dram_tensor("w3a", [H + 1, H], F16, kind="ExternalInput").ap()
    wo_d = nc.dram_tensor("wout", [H, 1], F16, kind="ExternalInput").ap()
    cb_d = nc.dram_tensor("cb", [128, 12], F32, kind="ExternalInput").ap()
    id_d = nc.dram_tensor("ident", [128, 128], F32, kind="ExternalInput").ap()
    q_d = nc.dram_tensor("q", [1, bc], F32, kind="ExternalOutput").ap()

    with tile.TileContext(nc) as tc:
        _emit(tc, ntiles, bout, newton2,
              zr_d, zt_d, w1_d, w2_d, w3_d, wo_d, cb_d, id_d, q_d)

    nc.compile()
    return nc


def _emit(tc, ntiles, bout, newton2,
          zr_d, zt_d, w1_d, w2_d, w3_d, wo_d, cb_d, id_d, q_d):
    nc = tc.nc
    with ExitStack() as ctx:
        wp = ctx.enter_context(tc.tile_pool(name="wp", bufs=1))
        zt_p = ctx.enter_context(tc.tile_pool(name="ztp", bufs=2))
        zr_p = ctx.enter_context(tc.tile_pool(name="zrp", bufs=2))
        h_p = ctx.enter_context(tc.tile_pool(name="hp", bufs=2))
        u_p = ctx.enter_context(tc.tile_pool(name="up", bufs=2))
        sq_p = ctx.enter_context(tc.tile_pool(name="sqp", bufs=2))
        st_p = ctx.enter_context(tc.tile_pool(name="stp", bufs=2))
        ps_y = ctx.enter_context(tc.tile_pool(name="psy", bufs=1, space="PSUM"))
        ps_sb = ctx.enter_context(tc.tile_pool(name="pssb", bufs=1, space="PSUM"))
        ps_b = ctx.enter_context(tc.tile_pool(name="psb", bufs=1, space="PSUM"))
        ps_m = ctx.enter_context(tc.tile_pool(name="psm", bufs=1, space="PSUM"))

        # ---- persistent constants / weights ----
        w1 = []
        for k in range(K1):
            rows = 128 if k < 16 else K1_LAST + 1
            t = wp.tile([rows, H], F16, tag=f"w1_{k}")
            nc.sync.dma_start(out=t[:, :], in_=w1_d[k * 128:k * 128 + rows, :])
            w1.append(t)
        w2, w3 = [], []
        for name, wd, lst in (("w2", w2_d, w2), ("w3", w3_d, w3)):
            for k in range(4):
                t = wp.tile([128, H], F16, tag=f"{name}_{k}")
                nc.sync.dma_start(out=t[:, :], in_=wd[k * 128:(k + 1) * 128, :])
                lst.append(t)
            t = wp.tile([1, H], F16, tag=f"{name}_rs")
            nc.sync.dma_start(out=t[:, :], in_=wd[H:H + 1, :])
            lst.append(t)
        wo = wp.tile([128, 4], F16, tag="wo")
        for k in range(4):
            nc.sync.dma_start(out=wo[:, k:k + 1], in_=wo_d[k * 128:(k + 1) * 128, :])
        cb = wp.tile([128, 12], F32, tag="cb")
        nc.sync.dma_start(out=cb[:, :], in_=cb_d[:, :])
        ident = wp.tile([128, 128], F32, tag="ident")
        nc.sync.dma_start(out=ident[:, :], in_=id_d[:, :])
        ones1 = wp.tile([1, 128], F16, tag="ones1")
        nc.vector.memset(ones1[:, :], 1.0)
        onesn = wp.tile([128, 128], F16, tag="onesn")
        nc.vector.memset(onesn[:, :], -1.0 / H)
        onesp = wp.tile([128, 128], F16, tag="onesp")
        nc.vector.memset(onesp[:, :], 1.0 / H)
        boutT = wp.tile([1, 1], F32, tag="boutT")
        nc.vector.memset(boutT[:, :], bout)
        epscol = wp.tile([1, 128], F16, tag="epscol")
        nc.vector.memset(epscol[:, :], SEPS)
        epsrow = wp.tile([1, NT], F16, tag="epsrow")
        nc.vector.memset(epsrow[:, :], SEPS)
        qrow = wp.tile([1, ntiles * NT], F32, tag="qrow")

        def quake_seed(y0, v_ap):
            """y0 ~ 1/sqrt(v), 3.4% max err (DVE bit trick)."""
            nc.vector.tensor_scalar(
                out=y0[:, :].bitcast(I32), in0=v_ap.bitcast(I32),
                scalar1=1, scalar2=None, op0=OP.logical_shift_right)
            nc.vector.tensor_scalar(
                out=y0[:, :].bitcast(I32), in0=y0[:, :].bitcast(I32),
                scalar1=-1, scalar2=MAGIC, op0=OP.mult, op1=OP.add)

        def rsqrt_dve(out_ap, v_ap, pool, shape, tagp, iters):
            """out = 1/sqrt(v) via Quake seed + Newton, all on DVE."""
            y0 = pool.tile(shape, F32, tag=f"{tagp}_y0")
            a = pool.tile(shape, F32, tag=f"{tagp}_a")
            quake_seed(y0, v_ap)
            for it in range(iters):
                nc.vector.tensor_mul(a[:, :], y0[:, :], y0[:, :])
                nc.vector.tensor_mul(a[:, :], a[:, :], v_ap)
                nc.vector.tensor_scalar(
                    out=a[:, :], in0=a[:, :],
                    scalar1=-0.5, scalar2=1.5, op0=OP.mult, op1=OP.add)
                last = out_ap if it == iters - 1 else y0[:, :]
                nc.vector.tensor_mul(last, y0[:, :], a[:, :])

        def emit_dma(j):
            """Issue all HBM loads for tile j (prefetched one tile ahead)."""
            bs = j * NT
            zrt = zr_p.tile([128, 4, D], F16, tag="zrall")
            nc.sync.dma_start(out=zrt[:, :, :],
                              in_=zr_d[bs:bs + NT, :].rearrange("(c p) d -> p c d", c=4))
            ztmain = zt_p.tile([128, 16, NT], F16, tag="ztmain")
            nc.sync.dma_start(
                out=ztmain[:, :, :],
                in_=zt_d[0:2048, bs:bs + NT].rearrange("(k p) n -> p k n", k=16))
            zt17 = zt_p.tile([K1_LAST + 1, NT], F16, tag="zt17")
            nc.sync.dma_start(out=zt17[0:K1_LAST, :],
                              in_=zt_d[2048:2048 + K1_LAST, bs:bs + NT])
            return {"zrt": zrt, "ztmain": ztmain, "zt17": zt17}

        def emit_stats(t):
            """L1 stats chain for tile j (runs one tile ahead of compute):
            bn_stats -> (-mu, invs) -> transpose -> aug row + invs row."""
            zrt, zt17 = t["zrt"], t["zt17"]
            stats = st_p.tile([128, 4, 5, 6], F32, tag="bnst")
            mv4 = st_p.tile([128, 4, 2], F32, tag="mv4")
            for bch in range(4):
                zrv = zrt[:, bch, :].rearrange("p (n s) -> p n s", n=5)
                for i in range(5):
                    nc.vector.bn_stats(out=stats[:, bch, i, :], in_=zrv[:, i, :])
                nc.vector.bn_aggr(out=mv4[:, bch, :], in_=stats[:, bch, :, :])
            pt = st_p.tile([128, 4, 2], F32, tag="pt")
            nc.vector.tensor_scalar(out=pt[:, :, 0], in0=mv4[:, :, 0],
                                    scalar1=-1.0, scalar2=None, op0=OP.mult)
            vpe4 = st_p.tile([128, 4], F32, tag="vpe4")
            nc.vector.tensor_scalar(out=vpe4[:, :], in0=mv4[:, :, 1],
                                    scalar1=EPS, scalar2=None, op0=OP.add)
            rsqrt_dve(pt[:, :, 1], vpe4[:, :], st_p, [128, 4], "l1n", iters=1)
            invs1row = st_p.tile([1, NT], F32, tag="invs1row")
            for b4 in range(4):
                sl = slice(b4 * 128, (b4 + 1) * 128)
                ptm = ps_m.tile([1, 128], F32, tag="ptr")
                nc.tensor.transpose(out=ptm[:, :], in_=pt[:, b4, 0:1],
                                    identity=ident[:, :])
                nc.vector.tensor_copy(out=zt17[K1_LAST:K1_LAST + 1, sl],
                                      in_=ptm[0:1, :])
                pti = ps_m.tile([1, 128], F32, tag="ptr")
                nc.tensor.transpose(out=pti[:, :], in_=pt[:, b4, 1:2],
                                    identity=ident[:, :])
                nc.vector.tensor_copy(out=invs1row[0:1, sl],
                                      in_=pti[0:1, :])
            invs1b = st_p.tile([128, NT], F32, tag="invs1b")
            nc.gpsimd.partition_broadcast(invs1b[:, :], invs1row[0:1, :])
            t["invs1b"] = invs1b
            t["invs1row"] = invs1row

        cur = emit_dma(0)
        emit_stats(cur)
        for it in range(ntiles):
            bs = it * NT
            nxt = emit_dma(it + 1) if it + 1 < ntiles else None

            ztmain, zt17 = cur["ztmain"], cur["zt17"]
            invs1b = cur["invs1b"]
            zts = [ztmain[:, k, :] for k in range(16)] + [zt17[:, :]]

            def stats_feed(sb, k, ht):
                """Accumulate this evac'd chunk into the next layer's stats:
                s1b += -h/H (mean), s2b += h^2/H (+eps on the last chunk)."""
                s1b, s2b = sb
                nc.tensor.matmul(s1b[:, :], lhsT=onesn[:, :], rhs=ht[:, :],
                                 start=(k == 0), stop=(k == 3))
                sq = sq_p.tile([128, NT], F16, tag=f"sq{k}")
                nc.scalar.activation(sq[:, :], ht[:, :], AF.Square)
                nc.tensor.matmul(s2b[:, :], lhsT=onesp[:, :], rhs=sq[:, :],
                                 start=(k == 0), stop=False)
                if k == 3:
                    nc.tensor.matmul(s2b[:, :], lhsT=epscol[:, :], rhs=epsrow[:, :],
                                     start=False, stop=True)

            def ln_chain(sb, lname):
                """(-mu, invs) for the next layer from its s1b/s2b psums."""
                s1b, s2b = sb
                negmu = st_p.tile([1, NT], F16, tag=f"negmu_{lname}")
                nc.vector.tensor_copy(out=negmu[:, :], in_=s1b[0:1, :])
                musq = st_p.tile([128, NT], F32, tag="musq")
                nc.scalar.activation(musq[:, :], s1b[:, :], AF.Square)
                ve = st_p.tile([128, NT], F32, tag="ve")
                nc.vector.tensor_sub(ve[:, :], s2b[:, :], musq[:, :])
                y0 = st_p.tile([128, NT], F32, tag="ln_y0")
                quake_seed(y0, ve[:, :])
                y0sq = st_p.tile([128, NT], F32, tag="ln_y0sq")
                nc.scalar.activation(y0sq[:, :], y0[:, :], AF.Square)
                w_ = st_p.tile([128, NT], F32, tag="ln_w")
                nc.vector.tensor_mul(w_[:, :], y0sq[:, :], ve[:, :])
                nc.vector.tensor_scalar(out=w_[:, :], in0=w_[:, :],
                                        scalar1=-0.5, scalar2=1.5,
                                        op0=OP.mult, op1=OP.add)
                invsb = st_p.tile([128, NT], F32, tag="invsb_sb")
                nc.vector.tensor_mul(invsb[:, :], y0[:, :], w_[:, :])
                return negmu, invsb

            # ---- L1 matmuls + evac; L2 stats fed per evac'd chunk ----
            sb2 = (ps_sb.tile([128, NT], F32, tag="s1b", name="s1b"),
                   ps_sb.tile([128, NT], F32, tag="s2b", name="s2b"))
            h1 = []
            for m in range(4):
                py = ps_y.tile([128, NT], F32, tag=f"py{m}")
                msl = slice(m * 128, (m + 1) * 128)
                for k in range(K1):
                    nc.tensor.matmul(py[:, :], lhsT=w1[k][:, msl], rhs=zts[k],
                                     start=(k == 0), stop=(k == K1 - 1))
                u = u_p.tile([128, NT], F16, tag=f"u{m}")
                nc.vector.tensor_mul(u[:, :], py[:, :], invs1b[:, :])
                ht = h_p.tile([128, NT], F16, tag=f"h1_{m}")
                nc.scalar.activation(ht[:, :], u[:, :], AF.Tanh, bias=cb[:, m:m + 1])
                h1.append(ht)
                stats_feed(sb2, m, ht)

            # ---- L2 / L3 ----
            hcur = h1
            for li, (lname, wts) in enumerate((("l2", w2), ("l3", w3))):
                negmu, invsb = ln_chain(sb2, lname)
                last = li == 1
                if not last:
                    sb2 = (ps_sb.tile([128, NT], F32, tag="s1b", name="s1b"),
                           ps_sb.tile([128, NT], F32, tag="s2b", name="s2b"))
                else:
                    pq = ps_b.tile([1, NT], F32, tag="pq")
                hnew = []
                for m in range(4):
                    py = ps_y.tile([128, NT], F32, tag=f"py{m}")
                    msl = slice(m * 128, (m + 1) * 128)
                    for k in range(4):
                        nc.tensor.matmul(py[:, :], lhsT=wts[k][:, msl], rhs=hcur[k][:, :],
                                         start=(k == 0), stop=False)
                    nc.tensor.matmul(py[:, :], lhsT=wts[4][:, msl], rhs=negmu[:, :],
                                     start=False, stop=True)
                    u = u_p.tile([128, NT], F16, tag=f"u{m}")
                    nc.vector.tensor_mul(u[:, :], py[:, :], invsb[:, :])
                    ht = h_p.tile([128, NT], F16, tag=f"h_{lname}_{m}")
                    nc.scalar.activation(ht[:, :], u[:, :], AF.Tanh,
                                         bias=cb[:, 4 + 4 * li + m:5 + 4 * li + m])
                    hnew.append(ht)
                    if not last:
                        stats_feed(sb2, m, ht)
                    else:
                        # L4 accumulation rides each L3 evac
                        nc.tensor.matmul(pq[0:1, :], lhsT=wo[:, m:m + 1],
                                         rhs=ht[:, :], start=(m == 0), stop=(m == 3))
                hcur = hnew

            nc.scalar.activation(qrow[0:1, bs:bs + NT], pq[0:1, :], AF.Tanh,
                                 bias=boutT[:, :])

            if nxt is not None:
                emit_stats(nxt)
            cur = nxt

        nc.sync.dma_start(out=q_d[:, :], in_=qrow[:, :])


# ---------------- host side ----------------

def host_prep(x, a, g1, beta1, g2, beta2, g3, beta3,
              w1, b1, w2, b2, w3, b3, w_out, b_out):
    f16 = np.float16
    z = np.empty((x.shape[0], D), dtype=f16)
    np.multiply(x[:, :HALF], np.float32(1.0 / X_NORM), out=z[:, :HALF], casting="unsafe")
    np.multiply(x[:, HALF:], np.float32(1.0 / V_NORM), out=z[:, HALF:INPUT_DIM], casting="unsafe")
    z[:, INPUT_DIM:] = a.astype(f16)

    def fold(w, g):
        wg = (w.astype(np.float64) * g.astype(np.float64)[None, :])
        rs = wg.sum(axis=1)
        out = np.empty((w.shape[1] + 1, w.shape[0]), dtype=f16)
        out[:w.shape[1]] = wg.T.astype(f16)
        out[w.shape[1]] = rs.astype(f16)
        return out

    def cvec(w, beta, b):
        return (w.astype(np.float64) @ beta.astype(np.float64)
                + b.astype(np.float64)).astype(np.float32)

    w1a = fold(w1, g1)
    w2a = fold(w2, g2)
    w3a = fold(w3, g3)
    cb = np.empty((128, 12), np.float32)
    for li, (w, beta, b) in enumerate(((w1, beta1, b1), (w2, beta2, b2),
                                       (w3, beta3, b3))):
        c = cvec(w, beta, b)
        for m in range(4):
            cb[:, 4 * li + m] = c[m * 128:(m + 1) * 128]
    wout = w_out.T.astype(f16)
    bout = float(b_out[0])
    ident = np.eye(128, dtype=np.float32)
    return z, w1a, w2a, w3a, wout, bout, cb, ident


_NC_CACHE = {}


def _get_nc(bout, bc=BC):
    key = (round(bout, 10), bc)
    if key not in _NC_CACHE:
        _NC_CACHE[key] = build_nc(bout, bc)
    return _NC_CACHE[key]


def _in_maps(z, w1a, w2a, w3a, wout, cb, ident, bc=BC, ncores=NCORES):
    maps = []
    for c in range(ncores):
        zc = z[c * bc:(c + 1) * bc]
        maps.append({
            "zr": np.ascontiguousarray(zc),
            "zt": np.ascontiguousarray(zc.T),
            "w1a": w1a, "w2a": w2a, "w3a": w3a, "wout": wout,
            "cb": cb, "ident": ident,
        })
    return maps


def prep_for_bench(inputs):
    z, w1a, w2a, w3a, wout, bout, cb, ident = host_prep(**inputs)
    nc = _get_nc(bout)
    return nc, _in_maps(z, w1a, w2a, w3a, wout, cb, ident)


def kernel(**inputs):
    inputs = {k: np.asarray(v) for k, v in inputs.items()}
    nc, in_maps = prep_for_bench(inputs)
    res = run_bass_kernel_spmd(nc, in_maps, list(range(NCORES)))
    q = np.concatenate([res.results[c]["q"].reshape(BC, 1) for c in range(NCORES)],
                       axis=0).astype(np.float32)
    return q
